# revision 38
# baseline (speedup 1.0000x reference)
"""Trainium2 Bass kernel for nn_Downsample_Spa: sigma-conv + gaussian unfold downsample.

Math (per batch image, one NeuronCore each; batch of 8 -> 8 cores):
  xp = reflect_pad(x)                                  # [64,130,130]
  sigma[o,p] = clamp(BN(conv3x3(xp))[o,p], 1e-4)       # at stride-2 positions p only
  graw[o,p]  = exp(-0.5*d2[o]/sigma^2 - ln64) / sigma  # /64 guards fp16 range; cancels in the ratio
  out[c,p]   = sum_o graw[o,p]*xp[c,p+off(o)] / sum_o graw[o,p]

Design:
 - partitions = (row-half hh, channel c) = 128; host pre-pads (reflect) and stores x
   in fp16 with columns parity-split into 3 planes (w=2j / w=2j+1 / w=2j+2) so every
   tap is a step-1 AP (enables PE full rate + DVE 2x modes); ~0.5 ulp fp16 noise.
 - conv: 9 accumulating fp16 matmuls per 512-position block, block-diagonal weights
   (M=18 computes both row halves per N-stream). sigma in fp32 PSUM.
 - g pipeline (fp32): clamp via tensor_scalar(add bias, max eps), custom-DVE fast
   reciprocal, ACT Square + Exp (one table set), -> gb fp16.
 - unfold: per tap a one-hot fp16 matmul broadcasts gb across the 64 channel
   partitions; taps are packed 3/2-wide, copied to SBUF fp16 (ACT), multiplied with
   x (DVE 2x), pair-tree summed (fp16), normalized by 1/sum (10th all-ones matmul +
   fast reciprocal); the center tap stays fp32 end-to-end.
"""

import os
import sys

import numpy as np

if "/opt/trn_rl_repo" not in sys.path:
    sys.path.insert(0, "/opt/trn_rl_repo")

K = 3
BN_EPS = 1e-5
SIGMA_MIN = 1e-4
GSCALE_LN = float(np.log(64.0))   # graw scaled by 1/64 (folded into exp bias)
N, C, H, W = 8, 64, 128, 128
HO = WO = 64
HH = 2
RS = 65                  # padded-row slots per partition-half
HOC = 32
NBLK = 4
BR = HOC // NBLK         # 8 output rows per block
NPOS = BR * WO           # 512
NP2 = 2 * NPOS
PL = 3                   # x col-parity planes: w=2j / w=2j+1 / w=2j+2
JW = 66                  # j slots per plane (65 used max, 66 for alignment)
CR = 17                  # rows per DMA chunk tile (16 + 1 overlap)

# f32 consts tensor columns
_D2 = 0                  # -0.5*d2[o] per (hh,o)
_BC = 1                  # bn_bias - sigma_min
_LB = 2                  # exp bias: constant -ln(64) per partition
_NCC = 3

_STATE = {}


def _build_consts(conv_w, bn_gamma, bn_beta, bn_mean, bn_var):
    s = (bn_gamma / np.sqrt(bn_var + BN_EPS)).astype(np.float32)
    wf = conv_w.astype(np.float32) * s[:, None, None, None]           # [9,64,3,3]
    bias = (bn_beta - bn_mean * s).astype(np.float32)

    cst = np.zeros((18, _NCC), np.float32)
    d2 = np.array([(kk // 3 - 1) ** 2 + (kk % 3 - 1) ** 2 for kk in range(9)], np.float32)
    for hh in range(HH):
        cst[hh * 9:hh * 9 + 9, _D2] = -0.5 * d2
        cst[hh * 9:hh * 9 + 9, _BC] = bias - SIGMA_MIN
        cst[hh * 9:hh * 9 + 9, _LB] = -GSCALE_LN

    # conv weights, block-diagonal per tap: win[k=hh*64+c, tap*18 + hh*9+o]
    win = np.zeros((128, 9 * 18), np.float16)
    for tap in range(9):
        i, j = tap // 3, tap % 3
        for hh in range(HH):
            win[hh * 64:hh * 64 + 64, tap * 18 + hh * 9:tap * 18 + hh * 9 + 9] = \
                wf[:, :, i, j].T.astype(np.float16)

    # one-hot / ones broadcast weights: gin[k=hh*9+o, tap*128 + hh*64+c]
    import ml_dtypes
    gin = np.zeros((18, 10 * 128), ml_dtypes.bfloat16)
    for hh in range(HH):
        gin[hh * 9:hh * 9 + 9, 9 * 128 + hh * 64:9 * 128 + hh * 64 + 64] = 1.0
        for tap in range(9):
            gin[hh * 9 + tap, tap * 128 + hh * 64:tap * 128 + hh * 64 + 64] = 1.0
    return cst, win, gin


def _build_bass(for_sim=False):
    import concourse.bass as bass
    import concourse.tile as tile
    from concourse import mybir

    f32 = mybir.dt.float32
    f16 = mybir.dt.float16
    bf16 = mybir.dt.bfloat16
    MULT = mybir.AluOpType.mult
    ADD = mybir.AluOpType.add
    MAX = mybir.AluOpType.max
    AF = mybir.ActivationFunctionType

    if for_sim:
        nc = bass.Bass("TRN2", target_bir_lowering=False, detect_race_conditions=False)
    else:
        from concourse import bacc
        nc = bacc.Bacc()
    xin = nc.dram_tensor("xin", [128, RS, PL, JW], f16, kind="ExternalInput")
    cin = nc.dram_tensor("cin", [18, _NCC], f32, kind="ExternalInput")
    win = nc.dram_tensor("win", [128, 9 * 18], f16, kind="ExternalInput")
    gin = nc.dram_tensor("gin", [18, 10 * 128], bf16, kind="ExternalInput")
    out = nc.dram_tensor("out", [128, HOC, WO], f32, kind="ExternalOutput")

    with tile.TileContext(nc) as tc:
        from contextlib import ExitStack
        with ExitStack() as ctx:
            big = ctx.enter_context(tc.tile_pool(name="big", bufs=1))
            gsb = ctx.enter_context(tc.tile_pool(name="gsb", bufs=3))
            y_p = ctx.enter_context(tc.tile_pool(name="y", bufs=3))
            ps_s = ctx.enter_context(tc.tile_pool(name="ps_s", bufs=2, space="PSUM"))
            ps_g = ctx.enter_context(tc.tile_pool(name="ps_g", bufs=2, space="PSUM"))

            ws = big.tile([128, 9 * 18], f16)
            nc.sync.dma_start(out=ws[:], in_=win[:])
            cs = big.tile([18, _NCC], f32)
            nc.gpsimd.dma_start(out=cs[:], in_=cin[:])
            gs = big.tile([18, 10 * 128], bf16)
            nc.gpsimd.dma_start(out=gs[:], in_=gin[:])

            xsk = []
            for blk in range(NBLK):
                xs = big.tile([128, CR, PL, JW], f16, tag=f"xs{blk}")
                nc.sync.dma_start(out=xs[:], in_=xin[:, 16 * blk:16 * blk + CR, :, :])
                xsk.append(xs)

            def xtap(tap, blk):
                # [128, 8, 64] fp16 step-1 view for tap (i,b) in block blk
                i, b = tap // 3, tap % 3
                return xsk[blk][:, i:i + 2 * BR - 1:2, b, 0:WO]

            def xtaps(i, pls, blk):
                # [128, nplanes, 8, 64] multi-tap view (planes outer, rows, cols)
                return xsk[blk][:, i:i + 2 * BR - 1:2, pls, 0:WO].transpose([0, 2, 1, 3])

            # ---- PE warm-up: dummy matmuls on the (early-arriving) weights tile
            # during the input-DMA wait, so HAM un-throttles before the conv ----
            wu = ps_s.tile([18, NPOS], f32, tag="sig")
            for _ in range(12):
                nc.tensor.matmul(wu[:, 0:162], ws[:, 0:18], ws[:, 0:162],
                                 start=True, stop=True)

            # ---- conv: per-block sigma [18,512] in PSUM, clamped to a [18,1024] SBUF half ----
            sigh = []
            for h in range(2):
                sc = gsb.tile([18, NP2], f32, tag=f"sc{h}")
                for sub in range(2):
                    blk = 2 * h + sub
                    sig = ps_s.tile([18, NPOS], f32, tag="sig")
                    for tap in range(9):
                        nc.tensor.matmul(
                            sig[:],
                            ws[:, tap * 18:(tap + 1) * 18],
                            xtap(tap, blk),
                            start=(tap == 0), stop=(tap == 8),
                        )
                    nc.vector.tensor_scalar(out=sc[:, sub * NPOS:(sub + 1) * NPOS],
                                            in0=sig[:],
                                            scalar1=cs[:, _BC:_BC + 1],
                                            scalar2=float(SIGMA_MIN),
                                            op0=ADD, op1=MAX)
                sigh.append(sc)

            def g_emit(sc):
                inv = gsb.tile([18, NP2], f32, tag="inv")
                nc.vector.reciprocal_approx_fast(out=inv[:], in_=sc[:])
                qt = gsb.tile([18, NP2], f32, tag="qt")
                nc.scalar.activation(out=qt[:], in_=inv[:], func=AF.Square)
                et = gsb.tile([18, NP2], f32, tag="et")
                nc.scalar.activation(out=et[:], in_=qt[:], func=AF.Exp,
                                     scale=cs[:, _D2:_D2 + 1],
                                     bias=cs[:, _LB:_LB + 1])
                gb = gsb.tile([18, NP2], bf16, tag="gb")
                nc.vector.tensor_tensor(out=gb[:], in0=et[:], in1=inv[:], op=MULT)
                return gb

            def unfold_emit(blk, gbs):
                # normalizer (10th tap): Srep[(hh,c),p] = sum_o gb
                Srep = ps_g.tile([128, 3, NPOS], f32, tag="grep")
                nc.tensor.matmul(Srep[:, 0, :], gs[:, 9 * 128:10 * 128],
                                 gbs, start=True, stop=True)
                rr = gsb.tile([128, NPOS], f32, tag="rr")
                nc.vector.reciprocal_approx_fast(out=rr[:], in_=Srep[:, 0, :])

                # tap groups: row0 triple (0,1,2) / row1 (3,5)+center 4 / row2 triple (6,7,8)
                yt = y_p.tile([128, 8, BR, WO], f16, tag="yt")
                y4 = y_p.tile([128, BR, WO], f32, tag="y4")

                def tap_group(taps, slot0, i, pls):
                    g3 = ps_g.tile([128, 3, NPOS], f32, tag="grep")
                    for k, tap in enumerate(taps):
                        nc.tensor.matmul(g3[:, k, :], gs[:, tap * 128:(tap + 1) * 128],
                                         gbs, start=True, stop=True)
                    gc = y_p.tile([128, len(taps), BR, WO], f16, tag=f"gc{slot0}")
                    nc.scalar.activation(out=gc[:], in_=g3[:, 0:len(taps), :], func=AF.Copy)
                    nc.vector.tensor_tensor(
                        out=yt[:, slot0:slot0 + len(taps)],
                        in0=xtaps(i, pls, blk), in1=gc[:], op=MULT)

                tap_group((0, 1, 2), 0, 0, slice(0, 3))
                tap_group((6, 7, 8), 5, 2, slice(0, 3))
                # row1: pair (3,5) at planes 0,2 + fp32 center (4)
                g2 = ps_g.tile([128, 3, NPOS], f32, tag="grep")
                nc.tensor.matmul(g2[:, 0, :], gs[:, 3 * 128:4 * 128], gbs, start=True, stop=True)
                nc.tensor.matmul(g2[:, 2, :], gs[:, 5 * 128:6 * 128], gbs, start=True, stop=True)
                nc.tensor.matmul(g2[:, 1, :], gs[:, 4 * 128:5 * 128], gbs, start=True, stop=True)
                nc.vector.tensor_tensor(out=yt[:, 3], in0=xtap(3, blk),
                                        in1=g2[:, 0, :], op=MULT)
                nc.vector.tensor_tensor(out=yt[:, 4], in0=xtap(5, blk),
                                        in1=g2[:, 2, :], op=MULT)
                nc.vector.tensor_tensor(out=y4[:], in0=xtap(4, blk), in1=g2[:, 1, :], op=MULT)

                # pair tree (fp16) + center + normalize
                t4 = y_p.tile([128, 4, BR, WO], f16, tag="t4")
                nc.vector.tensor_tensor(out=t4[:], in0=yt[:, 0:8:2], in1=yt[:, 1:8:2], op=ADD)
                late = blk == NBLK - 1
                t2 = y_p.tile([128, 2, BR, WO], f16, tag="t2")
                nc.vector.tensor_tensor(out=t2[:], in0=t4[:, 0:4:2], in1=t4[:, 1:4:2], op=ADD)
                t1 = y_p.tile([128, BR, WO], f16, tag="t1")
                (nc.vector if late else nc.gpsimd).tensor_tensor(out=t1[:], in0=t2[:, 0], in1=t2[:, 1], op=ADD)
                t0 = y_p.tile([128, BR, WO], f32, tag="t0")
                (nc.vector if late else nc.gpsimd).tensor_tensor(out=t0[:], in0=t1[:], in1=y4[:], op=ADD)
                acc = y_p.tile([128, BR, WO], f32, tag="acc")
                (nc.vector if late else nc.gpsimd).tensor_tensor(out=acc[:], in0=t0[:], in1=rr[:], op=MULT)
                nc.sync.dma_start(out=out[:, BR * blk:BR * (blk + 1), :], in_=acc[:])

            gb0 = g_emit(sigh[0])
            unfold_emit(0, gb0[:, 0:NPOS])
            gb1 = g_emit(sigh[1])
            unfold_emit(1, gb0[:, NPOS:NP2])
            unfold_emit(2, gb1[:, 0:NPOS])
            unfold_emit(3, gb1[:, NPOS:NP2])

    if not for_sim and not nc.is_finalized():
        nc.finalize()
    return nc


def _prep_inputs(x, conv_w, bn_gamma, bn_beta, bn_mean, bn_var):
    cst, win, gin = _build_consts(conv_w, bn_gamma, bn_beta, bn_mean, bn_var)
    xp = np.pad(np.asarray(x, np.float32), ((0, 0), (0, 0), (1, 1), (1, 1)),
                mode="reflect").astype(np.float16)                    # [8,64,130,130]
    in_maps = []
    for n in range(N):
        xc = np.concatenate([xp[n, :, 0:RS, :], xp[n, :, 64:64 + RS, :]], axis=0)
        xpl = np.zeros((128, RS, PL, JW), np.float16)
        xpl[:, :, 0, 0:65] = xc[:, :, 0:130:2]
        xpl[:, :, 1, 0:65] = xc[:, :, 1:130:2]
        xpl[:, :, 2, 0:64] = xc[:, :, 2:130:2]
        in_maps.append({"xin": xpl, "cin": cst, "win": win, "gin": gin})
    return in_maps


def _gather(results):
    out = np.empty((N, C, HO, WO), np.float32)
    for n in range(N):
        d = results[n]["out"]
        out[n, :, 0:HOC, :] = d[0:64]
        out[n, :, HOC:, :] = d[64:128]
    return out


def _enable_axon_trace():
    """Register the NTFF profile hook that this image's antenv lacks."""
    if _STATE.get("trace_hooked"):
        return
    import types
    import antenv
    from concourse import bass_utils
    mod = types.ModuleType("antenv.axon_hooks")
    mod._hook = None
    mod.set_axon_ntff_profile_hook = lambda h: setattr(mod, "_hook", h)
    mod.get_axon_ntff_profile_hook = lambda: mod._hook
    sys.modules["antenv.axon_hooks"] = mod
    antenv.axon_hooks = mod
    from trn_agent_boot.trn_boot import _ntff_profile_via_ctypes
    mod._hook = _ntff_profile_via_ctypes("/opt/axon/libaxon_pjrt.so")
    bass_utils.upload_artifacts = lambda tmpdir: tmpdir
    _STATE["trace_hooked"] = True


def run(x, conv_w, bn_gamma, bn_beta, bn_mean, bn_var, trace=False):
    from concourse.bass_utils import run_bass_kernel_spmd
    if trace:
        _enable_axon_trace()
    if "nc" not in _STATE:
        _STATE["nc"] = _build_bass()
    in_maps = _prep_inputs(x, conv_w, bn_gamma, bn_beta, bn_mean, bn_var)
    res = run_bass_kernel_spmd(_STATE["nc"], in_maps, list(range(N)), trace=trace)
    _STATE["last"] = res
    return _gather(res.results)


def kernel(x, conv_w, bn_gamma, bn_beta, bn_mean, bn_var):
    return run(x, conv_w, bn_gamma, bn_beta, bn_mean, bn_var,
               trace=bool(int(os.environ.get("KERNEL_TRACE", "0"))))


# revision 39
# speedup vs baseline: 1.0082x; 1.0082x over previous
"""Trainium2 Bass kernel for nn_Downsample_Spa: sigma-conv + gaussian unfold downsample.

Math (per batch image, one NeuronCore each; batch of 8 -> 8 cores):
  xp = reflect_pad(x)                                  # [64,130,130]
  sigma[o,p] = clamp(BN(conv3x3(xp))[o,p], 1e-4)       # at stride-2 positions p only
  graw[o,p]  = exp(-0.5*d2[o]/sigma^2 - ln64) / sigma  # /64 guards fp16 range; cancels in the ratio
  out[c,p]   = sum_o graw[o,p]*xp[c,p+off(o)] / sum_o graw[o,p]

Design:
 - partitions = (row-half hh, channel c) = 128; host pre-pads (reflect) and stores x
   in fp16 with columns parity-split into 3 planes (w=2j / w=2j+1 / w=2j+2) so every
   tap is a step-1 AP (enables PE full rate + DVE 2x modes); ~0.5 ulp fp16 noise.
 - conv: 9 accumulating fp16 matmuls per 512-position block, block-diagonal weights
   (M=18 computes both row halves per N-stream). sigma in fp32 PSUM.
 - g pipeline (fp32): clamp via tensor_scalar(add bias, max eps), custom-DVE fast
   reciprocal, ACT Square + Exp (one table set), -> gb fp16.
 - unfold: per tap a one-hot fp16 matmul broadcasts gb across the 64 channel
   partitions; taps are packed 3/2-wide, copied to SBUF fp16 (ACT), multiplied with
   x (DVE 2x), pair-tree summed (fp16), normalized by 1/sum (10th all-ones matmul +
   fast reciprocal); the center tap stays fp32 end-to-end.
"""

import os
import sys

import numpy as np

if "/opt/trn_rl_repo" not in sys.path:
    sys.path.insert(0, "/opt/trn_rl_repo")

K = 3
BN_EPS = 1e-5
SIGMA_MIN = 1e-4
GSCALE_LN = float(np.log(64.0))   # graw scaled by 1/64 (folded into exp bias)
N, C, H, W = 8, 64, 128, 128
HO = WO = 64
HH = 2
RS = 65                  # padded-row slots per partition-half
HOC = 32
NBLK = 4
BR = HOC // NBLK         # 8 output rows per block
NPOS = BR * WO           # 512
NP2 = 2 * NPOS
PL = 3                   # x col-parity planes: w=2j / w=2j+1 / w=2j+2
JW = 66                  # j slots per plane (65 used max, 66 for alignment)
CR = 17                  # rows per DMA chunk tile (16 + 1 overlap)

# f32 consts tensor columns
_D2 = 0                  # -0.5*d2[o] per (hh,o)
_BC = 1                  # bn_bias - sigma_min
_LB = 2                  # exp bias: constant -ln(64) per partition
_NCC = 3

_STATE = {}


def _build_consts(conv_w, bn_gamma, bn_beta, bn_mean, bn_var):
    s = (bn_gamma / np.sqrt(bn_var + BN_EPS)).astype(np.float32)
    wf = conv_w.astype(np.float32) * s[:, None, None, None]           # [9,64,3,3]
    bias = (bn_beta - bn_mean * s).astype(np.float32)

    cst = np.zeros((18, _NCC), np.float32)
    d2 = np.array([(kk // 3 - 1) ** 2 + (kk % 3 - 1) ** 2 for kk in range(9)], np.float32)
    for hh in range(HH):
        cst[hh * 9:hh * 9 + 9, _D2] = -0.5 * d2
        cst[hh * 9:hh * 9 + 9, _BC] = bias - SIGMA_MIN
        cst[hh * 9:hh * 9 + 9, _LB] = -GSCALE_LN

    # conv weights, block-diagonal per tap: win[k=hh*64+c, tap*18 + hh*9+o]
    win = np.zeros((128, 9 * 18), np.float16)
    for tap in range(9):
        i, j = tap // 3, tap % 3
        for hh in range(HH):
            win[hh * 64:hh * 64 + 64, tap * 18 + hh * 9:tap * 18 + hh * 9 + 9] = \
                wf[:, :, i, j].T.astype(np.float16)

    # one-hot / ones broadcast weights: gin[k=hh*9+o, tap*128 + hh*64+c]
    import ml_dtypes
    gin = np.zeros((18, 10 * 128), ml_dtypes.bfloat16)
    for hh in range(HH):
        gin[hh * 9:hh * 9 + 9, 9 * 128 + hh * 64:9 * 128 + hh * 64 + 64] = 1.0
        for tap in range(9):
            gin[hh * 9 + tap, tap * 128 + hh * 64:tap * 128 + hh * 64 + 64] = 1.0
    return cst, win, gin


def _build_bass(for_sim=False):
    import concourse.bass as bass
    import concourse.tile as tile
    from concourse import mybir

    f32 = mybir.dt.float32
    f16 = mybir.dt.float16
    bf16 = mybir.dt.bfloat16
    MULT = mybir.AluOpType.mult
    ADD = mybir.AluOpType.add
    MAX = mybir.AluOpType.max
    AF = mybir.ActivationFunctionType

    if for_sim:
        nc = bass.Bass("TRN2", target_bir_lowering=False, detect_race_conditions=False)
    else:
        from concourse import bacc
        nc = bacc.Bacc()
    xin = nc.dram_tensor("xin", [128, RS, PL, JW], f16, kind="ExternalInput")
    cin = nc.dram_tensor("cin", [18, _NCC], f32, kind="ExternalInput")
    win = nc.dram_tensor("win", [128, 9 * 18], f16, kind="ExternalInput")
    gin = nc.dram_tensor("gin", [18, 10 * 128], bf16, kind="ExternalInput")
    out = nc.dram_tensor("out", [128, HOC, WO], f32, kind="ExternalOutput")

    with tile.TileContext(nc) as tc:
        from contextlib import ExitStack
        with ExitStack() as ctx:
            big = ctx.enter_context(tc.tile_pool(name="big", bufs=1))
            gsb = ctx.enter_context(tc.tile_pool(name="gsb", bufs=3))
            y_p = ctx.enter_context(tc.tile_pool(name="y", bufs=3))
            ps_s = ctx.enter_context(tc.tile_pool(name="ps_s", bufs=2, space="PSUM"))
            ps_g = ctx.enter_context(tc.tile_pool(name="ps_g", bufs=2, space="PSUM"))

            ws = big.tile([128, 9 * 18], f16)
            nc.sync.dma_start(out=ws[:], in_=win[:])
            cs = big.tile([18, _NCC], f32)
            nc.gpsimd.dma_start(out=cs[:], in_=cin[:])
            gs = big.tile([18, 10 * 128], bf16)
            nc.gpsimd.dma_start(out=gs[:], in_=gin[:])

            xsk = []
            for blk in range(NBLK):
                xs = big.tile([128, CR, PL, JW], f16, tag=f"xs{blk}")
                nc.sync.dma_start(out=xs[:], in_=xin[:, 16 * blk:16 * blk + CR, :, :])
                xsk.append(xs)

            def xtap(tap, blk):
                # [128, 8, 64] fp16 step-1 view for tap (i,b) in block blk
                i, b = tap // 3, tap % 3
                return xsk[blk][:, i:i + 2 * BR - 1:2, b, 0:WO]

            def xtaps(i, pls, blk):
                # [128, nplanes, 8, 64] multi-tap view (planes outer, rows, cols)
                return xsk[blk][:, i:i + 2 * BR - 1:2, pls, 0:WO].transpose([0, 2, 1, 3])

            # ---- PE warm-up: dummy matmuls on the (early-arriving) weights tile
            # during the input-DMA wait, so HAM un-throttles before the conv ----
            wu = ps_s.tile([18, NPOS], f32, tag="sig")
            for _ in range(12):
                nc.tensor.matmul(wu[:, 0:162], ws[:, 0:18], ws[:, 0:162],
                                 start=True, stop=True)

            # ---- conv: per-block sigma [18,512] in PSUM, clamped to a [18,1024] SBUF half ----
            sigh = []
            for h in range(2):
                sc = gsb.tile([18, NP2], f32, tag=f"sc{h}")
                for sub in range(2):
                    blk = 2 * h + sub
                    sig = ps_s.tile([18, NPOS], f32, tag="sig")
                    for tap in range(9):
                        nc.tensor.matmul(
                            sig[:],
                            ws[:, tap * 18:(tap + 1) * 18],
                            xtap(tap, blk),
                            start=(tap == 0), stop=(tap == 8),
                        )
                    nc.vector.tensor_scalar(out=sc[:, sub * NPOS:(sub + 1) * NPOS],
                                            in0=sig[:],
                                            scalar1=cs[:, _BC:_BC + 1],
                                            scalar2=float(SIGMA_MIN),
                                            op0=ADD, op1=MAX)
                sigh.append(sc)

            def g_emit(sc):
                inv = gsb.tile([18, NP2], f32, tag="inv")
                nc.vector.reciprocal_approx_fast(out=inv[:], in_=sc[:])
                qt = gsb.tile([18, NP2], f32, tag="qt")
                nc.scalar.activation(out=qt[:], in_=inv[:], func=AF.Square)
                et = gsb.tile([18, NP2], f32, tag="et")
                nc.scalar.activation(out=et[:], in_=qt[:], func=AF.Exp,
                                     scale=cs[:, _D2:_D2 + 1],
                                     bias=cs[:, _LB:_LB + 1])
                gb = gsb.tile([18, NP2], bf16, tag="gb")
                nc.vector.tensor_tensor(out=gb[:], in0=et[:], in1=inv[:], op=MULT)
                return gb

            def unfold_emit(blk, gbs):
                # normalizer (10th tap): Srep[(hh,c),p] = sum_o gb
                Srep = ps_g.tile([128, 3, NPOS], f32, tag="grep")
                nc.tensor.matmul(Srep[:, 0, :], gs[:, 9 * 128:10 * 128],
                                 gbs, start=True, stop=True)
                rr = gsb.tile([128, NPOS], f32, tag="rr")
                nc.vector.reciprocal_approx_fast(out=rr[:], in_=Srep[:, 0, :])

                # tap groups: row0 triple (0,1,2) / row1 (3,5)+center 4 / row2 triple (6,7,8)
                yt = y_p.tile([128, 8, BR, WO], f16, tag="yt")
                y4 = y_p.tile([128, BR, WO], f32, tag="y4")

                def tap_group(taps, slot0, i, pls):
                    g3 = ps_g.tile([128, 3, NPOS], f32, tag="grep")
                    for k, tap in enumerate(taps):
                        nc.tensor.matmul(g3[:, k, :], gs[:, tap * 128:(tap + 1) * 128],
                                         gbs, start=True, stop=True)
                    gc = y_p.tile([128, len(taps), BR, WO], f16, tag=f"gc{slot0}")
                    nc.scalar.activation(out=gc[:], in_=g3[:, 0:len(taps), :], func=AF.Copy)
                    nc.vector.tensor_tensor(
                        out=yt[:, slot0:slot0 + len(taps)],
                        in0=xtaps(i, pls, blk), in1=gc[:], op=MULT)

                tap_group((0, 1, 2), 0, 0, slice(0, 3))
                tap_group((6, 7, 8), 5, 2, slice(0, 3))
                # row1: pair (3,5) at planes 0,2 + fp32 center (4)
                g2 = ps_g.tile([128, 3, NPOS], f32, tag="grep")
                nc.tensor.matmul(g2[:, 0, :], gs[:, 3 * 128:4 * 128], gbs, start=True, stop=True)
                nc.tensor.matmul(g2[:, 2, :], gs[:, 5 * 128:6 * 128], gbs, start=True, stop=True)
                nc.tensor.matmul(g2[:, 1, :], gs[:, 4 * 128:5 * 128], gbs, start=True, stop=True)
                gc2 = y_p.tile([128, 2, BR, WO], f16, tag="gc2p")
                nc.scalar.activation(out=gc2[:], in_=g2[:, 0:3:2, :], func=AF.Copy)
                nc.vector.tensor_tensor(out=yt[:, 3:5], in0=xtaps(1, slice(0, 3, 2), blk),
                                        in1=gc2[:], op=MULT)
                nc.vector.tensor_tensor(out=y4[:], in0=xtap(4, blk), in1=g2[:, 1, :], op=MULT)

                # pair tree (fp16) + center + normalize
                t4 = y_p.tile([128, 4, BR, WO], f16, tag="t4")
                nc.vector.tensor_tensor(out=t4[:], in0=yt[:, 0:8:2], in1=yt[:, 1:8:2], op=ADD)
                late = blk == NBLK - 1
                t2 = y_p.tile([128, 2, BR, WO], f16, tag="t2")
                nc.vector.tensor_tensor(out=t2[:], in0=t4[:, 0:4:2], in1=t4[:, 1:4:2], op=ADD)
                t1 = y_p.tile([128, BR, WO], f16, tag="t1")
                (nc.vector if late else nc.gpsimd).tensor_tensor(out=t1[:], in0=t2[:, 0], in1=t2[:, 1], op=ADD)
                t0 = y_p.tile([128, BR, WO], f32, tag="t0")
                (nc.vector if late else nc.gpsimd).tensor_tensor(out=t0[:], in0=t1[:], in1=y4[:], op=ADD)
                acc = y_p.tile([128, BR, WO], f32, tag="acc")
                (nc.vector if late else nc.gpsimd).tensor_tensor(out=acc[:], in0=t0[:], in1=rr[:], op=MULT)
                nc.sync.dma_start(out=out[:, BR * blk:BR * (blk + 1), :], in_=acc[:])

            gb0 = g_emit(sigh[0])
            unfold_emit(0, gb0[:, 0:NPOS])
            gb1 = g_emit(sigh[1])
            unfold_emit(1, gb0[:, NPOS:NP2])
            unfold_emit(2, gb1[:, 0:NPOS])
            unfold_emit(3, gb1[:, NPOS:NP2])

    if not for_sim and not nc.is_finalized():
        nc.finalize()
    return nc


def _prep_inputs(x, conv_w, bn_gamma, bn_beta, bn_mean, bn_var):
    cst, win, gin = _build_consts(conv_w, bn_gamma, bn_beta, bn_mean, bn_var)
    xp = np.pad(np.asarray(x, np.float32), ((0, 0), (0, 0), (1, 1), (1, 1)),
                mode="reflect").astype(np.float16)                    # [8,64,130,130]
    in_maps = []
    for n in range(N):
        xc = np.concatenate([xp[n, :, 0:RS, :], xp[n, :, 64:64 + RS, :]], axis=0)
        xpl = np.zeros((128, RS, PL, JW), np.float16)
        xpl[:, :, 0, 0:65] = xc[:, :, 0:130:2]
        xpl[:, :, 1, 0:65] = xc[:, :, 1:130:2]
        xpl[:, :, 2, 0:64] = xc[:, :, 2:130:2]
        in_maps.append({"xin": xpl, "cin": cst, "win": win, "gin": gin})
    return in_maps


def _gather(results):
    out = np.empty((N, C, HO, WO), np.float32)
    for n in range(N):
        d = results[n]["out"]
        out[n, :, 0:HOC, :] = d[0:64]
        out[n, :, HOC:, :] = d[64:128]
    return out


def _enable_axon_trace():
    """Register the NTFF profile hook that this image's antenv lacks."""
    if _STATE.get("trace_hooked"):
        return
    import types
    import antenv
    from concourse import bass_utils
    mod = types.ModuleType("antenv.axon_hooks")
    mod._hook = None
    mod.set_axon_ntff_profile_hook = lambda h: setattr(mod, "_hook", h)
    mod.get_axon_ntff_profile_hook = lambda: mod._hook
    sys.modules["antenv.axon_hooks"] = mod
    antenv.axon_hooks = mod
    from trn_agent_boot.trn_boot import _ntff_profile_via_ctypes
    mod._hook = _ntff_profile_via_ctypes("/opt/axon/libaxon_pjrt.so")
    bass_utils.upload_artifacts = lambda tmpdir: tmpdir
    _STATE["trace_hooked"] = True


def run(x, conv_w, bn_gamma, bn_beta, bn_mean, bn_var, trace=False):
    from concourse.bass_utils import run_bass_kernel_spmd
    if trace:
        _enable_axon_trace()
    if "nc" not in _STATE:
        _STATE["nc"] = _build_bass()
    in_maps = _prep_inputs(x, conv_w, bn_gamma, bn_beta, bn_mean, bn_var)
    res = run_bass_kernel_spmd(_STATE["nc"], in_maps, list(range(N)), trace=trace)
    _STATE["last"] = res
    return _gather(res.results)


def kernel(x, conv_w, bn_gamma, bn_beta, bn_mean, bn_var):
    return run(x, conv_w, bn_gamma, bn_beta, bn_mean, bn_var,
               trace=bool(int(os.environ.get("KERNEL_TRACE", "0"))))


# revision 40
# speedup vs baseline: 1.0154x; 1.0071x over previous
"""Trainium2 Bass kernel for nn_Downsample_Spa: sigma-conv + gaussian unfold downsample.

Math (per batch image, one NeuronCore each; batch of 8 -> 8 cores):
  xp = reflect_pad(x)                                  # [64,130,130]
  sigma[o,p] = clamp(BN(conv3x3(xp))[o,p], 1e-4)       # at stride-2 positions p only
  graw[o,p]  = exp(-0.5*d2[o]/sigma^2 - ln64) / sigma  # /64 guards fp16 range; cancels in the ratio
  out[c,p]   = sum_o graw[o,p]*xp[c,p+off(o)] / sum_o graw[o,p]

Design:
 - partitions = (row-half hh, channel c) = 128; host pre-pads (reflect) and stores x
   in fp16 with columns parity-split into 3 planes (w=2j / w=2j+1 / w=2j+2) so every
   tap is a step-1 AP (enables PE full rate + DVE 2x modes); ~0.5 ulp fp16 noise.
 - conv: 9 accumulating fp16 matmuls per 512-position block, block-diagonal weights
   (M=18 computes both row halves per N-stream). sigma in fp32 PSUM.
 - g pipeline (fp32): clamp via tensor_scalar(add bias, max eps), custom-DVE fast
   reciprocal, ACT Square + Exp (one table set), -> gb fp16.
 - unfold: per tap a one-hot fp16 matmul broadcasts gb across the 64 channel
   partitions; taps are packed 3/2-wide, copied to SBUF fp16 (ACT), multiplied with
   x (DVE 2x), pair-tree summed (fp16), normalized by 1/sum (10th all-ones matmul +
   fast reciprocal); the center tap stays fp32 end-to-end.
"""

import os
import sys

import numpy as np

if "/opt/trn_rl_repo" not in sys.path:
    sys.path.insert(0, "/opt/trn_rl_repo")

K = 3
BN_EPS = 1e-5
SIGMA_MIN = 1e-4
GSCALE_LN = float(np.log(64.0))   # graw scaled by 1/64 (folded into exp bias)
N, C, H, W = 8, 64, 128, 128
HO = WO = 64
HH = 2
RS = 65                  # padded-row slots per partition-half
HOC = 32
NBLK = 4
BR = HOC // NBLK         # 8 output rows per block
NPOS = BR * WO           # 512
NP2 = 2 * NPOS
PL = 3                   # x col-parity planes: w=2j / w=2j+1 / w=2j+2
JW = 66                  # j slots per plane (65 used max, 66 for alignment)
CR = 17                  # rows per DMA chunk tile (16 + 1 overlap)

# f32 consts tensor columns
_D2 = 0                  # -0.5*d2[o] per (hh,o)
_BC = 1                  # bn_bias - sigma_min
_LB = 2                  # exp bias: constant -ln(64) per partition
_NCC = 3

_STATE = {}


def _build_consts(conv_w, bn_gamma, bn_beta, bn_mean, bn_var):
    s = (bn_gamma / np.sqrt(bn_var + BN_EPS)).astype(np.float32)
    wf = conv_w.astype(np.float32) * s[:, None, None, None]           # [9,64,3,3]
    bias = (bn_beta - bn_mean * s).astype(np.float32)

    cst = np.zeros((18, _NCC), np.float32)
    d2 = np.array([(kk // 3 - 1) ** 2 + (kk % 3 - 1) ** 2 for kk in range(9)], np.float32)
    for hh in range(HH):
        cst[hh * 9:hh * 9 + 9, _D2] = -0.5 * d2
        cst[hh * 9:hh * 9 + 9, _BC] = bias - SIGMA_MIN
        cst[hh * 9:hh * 9 + 9, _LB] = -GSCALE_LN

    # conv weights, block-diagonal per tap: win[k=hh*64+c, tap*18 + hh*9+o]
    win = np.zeros((128, 9 * 18), np.float16)
    for tap in range(9):
        i, j = tap // 3, tap % 3
        for hh in range(HH):
            win[hh * 64:hh * 64 + 64, tap * 18 + hh * 9:tap * 18 + hh * 9 + 9] = \
                wf[:, :, i, j].T.astype(np.float16)

    # one-hot / ones broadcast weights: gin[k=hh*9+o, tap*128 + hh*64+c]
    import ml_dtypes
    gin = np.zeros((18, 10 * 128), ml_dtypes.bfloat16)
    for hh in range(HH):
        gin[hh * 9:hh * 9 + 9, 9 * 128 + hh * 64:9 * 128 + hh * 64 + 64] = 1.0
        for tap in range(9):
            gin[hh * 9 + tap, tap * 128 + hh * 64:tap * 128 + hh * 64 + 64] = 1.0
    return cst, win, gin


def _build_bass(for_sim=False):
    import concourse.bass as bass
    import concourse.tile as tile
    from concourse import mybir

    f32 = mybir.dt.float32
    f16 = mybir.dt.float16
    bf16 = mybir.dt.bfloat16
    MULT = mybir.AluOpType.mult
    ADD = mybir.AluOpType.add
    MAX = mybir.AluOpType.max
    AF = mybir.ActivationFunctionType

    if for_sim:
        nc = bass.Bass("TRN2", target_bir_lowering=False, detect_race_conditions=False)
    else:
        from concourse import bacc
        nc = bacc.Bacc()
    xin = nc.dram_tensor("xin", [128, RS, PL, JW], f16, kind="ExternalInput")
    cin = nc.dram_tensor("cin", [18, _NCC], f32, kind="ExternalInput")
    win = nc.dram_tensor("win", [128, 9 * 18], f16, kind="ExternalInput")
    gin = nc.dram_tensor("gin", [18, 10 * 128], bf16, kind="ExternalInput")
    out = nc.dram_tensor("out", [128, HOC, WO], f32, kind="ExternalOutput")

    with tile.TileContext(nc) as tc:
        from contextlib import ExitStack
        with ExitStack() as ctx:
            big = ctx.enter_context(tc.tile_pool(name="big", bufs=1))
            gsb = ctx.enter_context(tc.tile_pool(name="gsb", bufs=3))
            y_p = ctx.enter_context(tc.tile_pool(name="y", bufs=3))
            ps_s = ctx.enter_context(tc.tile_pool(name="ps_s", bufs=2, space="PSUM"))
            ps_g = ctx.enter_context(tc.tile_pool(name="ps_g", bufs=2, space="PSUM"))

            ws = big.tile([128, 9 * 18], f16)
            nc.sync.dma_start(out=ws[:], in_=win[:])
            cs = big.tile([18, _NCC], f32)
            nc.gpsimd.dma_start(out=cs[:], in_=cin[:])
            gs = big.tile([18, 10 * 128], bf16)
            nc.gpsimd.dma_start(out=gs[:], in_=gin[:])

            xsk = []
            for blk in range(NBLK):
                xs = big.tile([128, CR, PL, JW], f16, tag=f"xs{blk}")
                nc.sync.dma_start(out=xs[:], in_=xin[:, 16 * blk:16 * blk + CR, :, :])
                xsk.append(xs)

            def xtap(tap, blk):
                # [128, 8, 64] fp16 step-1 view for tap (i,b) in block blk
                i, b = tap // 3, tap % 3
                return xsk[blk][:, i:i + 2 * BR - 1:2, b, 0:WO]

            def xtaps(i, pls, blk):
                # [128, nplanes, 8, 64] multi-tap view (planes outer, rows, cols)
                return xsk[blk][:, i:i + 2 * BR - 1:2, pls, 0:WO].transpose([0, 2, 1, 3])

            # ---- PE warm-up: dummy matmuls on the (early-arriving) weights tile
            # during the input-DMA wait, so HAM un-throttles before the conv ----
            wu = ps_s.tile([18, NPOS], f32, tag="sig")
            for _ in range(12):
                nc.tensor.matmul(wu[:, 0:162], ws[:, 0:18], ws[:, 0:162],
                                 start=True, stop=True)

            # ---- conv: per-block sigma [18,512] in PSUM, clamped to a [18,1024] SBUF half ----
            sigh = []
            for h in range(2):
                sc = gsb.tile([18, NP2], f32, tag=f"sc{h}")
                for sub in range(2):
                    blk = 2 * h + sub
                    sig = ps_s.tile([18, NPOS], f32, tag="sig")
                    for tap in range(9):
                        nc.tensor.matmul(
                            sig[:],
                            ws[:, tap * 18:(tap + 1) * 18],
                            xtap(tap, blk),
                            start=(tap == 0), stop=(tap == 8),
                        )
                    nc.vector.tensor_scalar(out=sc[:, sub * NPOS:(sub + 1) * NPOS],
                                            in0=sig[:],
                                            scalar1=cs[:, _BC:_BC + 1],
                                            scalar2=float(SIGMA_MIN),
                                            op0=ADD, op1=MAX)
                sigh.append(sc)

            def g_emit(sc):
                inv = gsb.tile([18, NP2], f32, tag="inv")
                nc.vector.reciprocal_approx_fast(out=inv[:], in_=sc[:])
                qt = gsb.tile([18, NP2], f32, tag="qt")
                nc.scalar.activation(out=qt[:], in_=inv[:], func=AF.Square)
                et = gsb.tile([18, NP2], f32, tag="et")
                nc.scalar.activation(out=et[:], in_=qt[:], func=AF.Exp,
                                     scale=cs[:, _D2:_D2 + 1],
                                     bias=cs[:, _LB:_LB + 1])
                gb = gsb.tile([18, NP2], bf16, tag="gb")
                nc.vector.tensor_tensor(out=gb[:], in0=et[:], in1=inv[:], op=MULT)
                return gb

            def unfold_emit(blk, gbs):
                # normalizer (10th tap): Srep[(hh,c),p] = sum_o gb
                Srep = ps_s.tile([128, NPOS], f32, tag="sig")
                nc.tensor.matmul(Srep[:], gs[:, 9 * 128:10 * 128],
                                 gbs, start=True, stop=True)
                rr = gsb.tile([128, NPOS], f32, tag="rr")
                nc.vector.reciprocal_approx_fast(out=rr[:], in_=Srep[:])

                # tap groups: row0 triple (0,1,2) / row1 (3,5)+center 4 / row2 triple (6,7,8)
                yt = y_p.tile([128, 8, BR, WO], f16, tag="yt")
                y4 = y_p.tile([128, BR, WO], f32, tag="y4")

                def tap_group(taps, slot0, i, pls):
                    g3 = ps_g.tile([128, 3, NPOS], f32, tag="grep")
                    for k, tap in enumerate(taps):
                        nc.tensor.matmul(g3[:, k, :], gs[:, tap * 128:(tap + 1) * 128],
                                         gbs, start=True, stop=True)
                    gc = y_p.tile([128, len(taps), BR, WO], f16, tag=f"gc{slot0}")
                    nc.scalar.activation(out=gc[:], in_=g3[:, 0:len(taps), :], func=AF.Copy)
                    nc.vector.tensor_tensor(
                        out=yt[:, slot0:slot0 + len(taps)],
                        in0=xtaps(i, pls, blk), in1=gc[:], op=MULT)

                tap_group((0, 1, 2), 0, 0, slice(0, 3))
                tap_group((6, 7, 8), 5, 2, slice(0, 3))
                # row1: pair (3,5) at planes 0,2 + fp32 center (4)
                g2 = ps_g.tile([128, 3, NPOS], f32, tag="grep")
                nc.tensor.matmul(g2[:, 0, :], gs[:, 3 * 128:4 * 128], gbs, start=True, stop=True)
                nc.tensor.matmul(g2[:, 2, :], gs[:, 5 * 128:6 * 128], gbs, start=True, stop=True)
                nc.tensor.matmul(g2[:, 1, :], gs[:, 4 * 128:5 * 128], gbs, start=True, stop=True)
                gc2 = y_p.tile([128, 2, BR, WO], f16, tag="gc2p")
                nc.scalar.activation(out=gc2[:], in_=g2[:, 0:3:2, :], func=AF.Copy)
                nc.vector.tensor_tensor(out=yt[:, 3:5], in0=xtaps(1, slice(0, 3, 2), blk),
                                        in1=gc2[:], op=MULT)
                nc.vector.tensor_tensor(out=y4[:], in0=xtap(4, blk), in1=g2[:, 1, :], op=MULT)

                # pair tree (fp16) + center + normalize
                t4 = y_p.tile([128, 4, BR, WO], f16, tag="t4")
                nc.vector.tensor_tensor(out=t4[:], in0=yt[:, 0:8:2], in1=yt[:, 1:8:2], op=ADD)
                late = blk == NBLK - 1
                t2 = y_p.tile([128, 2, BR, WO], f16, tag="t2")
                nc.vector.tensor_tensor(out=t2[:], in0=t4[:, 0:4:2], in1=t4[:, 1:4:2], op=ADD)
                t1 = y_p.tile([128, BR, WO], f16, tag="t1")
                (nc.vector if late else nc.gpsimd).tensor_tensor(out=t1[:], in0=t2[:, 0], in1=t2[:, 1], op=ADD)
                t0 = y_p.tile([128, BR, WO], f32, tag="t0")
                (nc.vector if late else nc.gpsimd).tensor_tensor(out=t0[:], in0=t1[:], in1=y4[:], op=ADD)
                acc = y_p.tile([128, BR, WO], f32, tag="acc")
                (nc.vector if late else nc.gpsimd).tensor_tensor(out=acc[:], in0=t0[:], in1=rr[:], op=MULT)
                nc.sync.dma_start(out=out[:, BR * blk:BR * (blk + 1), :], in_=acc[:])

            gb0 = g_emit(sigh[0])
            unfold_emit(0, gb0[:, 0:NPOS])
            gb1 = g_emit(sigh[1])
            unfold_emit(1, gb0[:, NPOS:NP2])
            unfold_emit(2, gb1[:, 0:NPOS])
            unfold_emit(3, gb1[:, NPOS:NP2])

    if not for_sim and not nc.is_finalized():
        nc.finalize()
    return nc


def _prep_inputs(x, conv_w, bn_gamma, bn_beta, bn_mean, bn_var):
    cst, win, gin = _build_consts(conv_w, bn_gamma, bn_beta, bn_mean, bn_var)
    xp = np.pad(np.asarray(x, np.float32), ((0, 0), (0, 0), (1, 1), (1, 1)),
                mode="reflect").astype(np.float16)                    # [8,64,130,130]
    in_maps = []
    for n in range(N):
        xc = np.concatenate([xp[n, :, 0:RS, :], xp[n, :, 64:64 + RS, :]], axis=0)
        xpl = np.zeros((128, RS, PL, JW), np.float16)
        xpl[:, :, 0, 0:65] = xc[:, :, 0:130:2]
        xpl[:, :, 1, 0:65] = xc[:, :, 1:130:2]
        xpl[:, :, 2, 0:64] = xc[:, :, 2:130:2]
        in_maps.append({"xin": xpl, "cin": cst, "win": win, "gin": gin})
    return in_maps


def _gather(results):
    out = np.empty((N, C, HO, WO), np.float32)
    for n in range(N):
        d = results[n]["out"]
        out[n, :, 0:HOC, :] = d[0:64]
        out[n, :, HOC:, :] = d[64:128]
    return out


def _enable_axon_trace():
    """Register the NTFF profile hook that this image's antenv lacks."""
    if _STATE.get("trace_hooked"):
        return
    import types
    import antenv
    from concourse import bass_utils
    mod = types.ModuleType("antenv.axon_hooks")
    mod._hook = None
    mod.set_axon_ntff_profile_hook = lambda h: setattr(mod, "_hook", h)
    mod.get_axon_ntff_profile_hook = lambda: mod._hook
    sys.modules["antenv.axon_hooks"] = mod
    antenv.axon_hooks = mod
    from trn_agent_boot.trn_boot import _ntff_profile_via_ctypes
    mod._hook = _ntff_profile_via_ctypes("/opt/axon/libaxon_pjrt.so")
    bass_utils.upload_artifacts = lambda tmpdir: tmpdir
    _STATE["trace_hooked"] = True


def run(x, conv_w, bn_gamma, bn_beta, bn_mean, bn_var, trace=False):
    from concourse.bass_utils import run_bass_kernel_spmd
    if trace:
        _enable_axon_trace()
    if "nc" not in _STATE:
        _STATE["nc"] = _build_bass()
    in_maps = _prep_inputs(x, conv_w, bn_gamma, bn_beta, bn_mean, bn_var)
    res = run_bass_kernel_spmd(_STATE["nc"], in_maps, list(range(N)), trace=trace)
    _STATE["last"] = res
    return _gather(res.results)


def kernel(x, conv_w, bn_gamma, bn_beta, bn_mean, bn_var):
    return run(x, conv_w, bn_gamma, bn_beta, bn_mean, bn_var,
               trace=bool(int(os.environ.get("KERNEL_TRACE", "0"))))
